# revision 28
# baseline (speedup 1.0000x reference)
"""Multi-head attention (B=4, S=2048, D=1024, H=16, dk=64) on 8 TRN2 cores.

Sharding: data-parallel over B (4 batches) x tensor-parallel over head
groups (2 groups of 8 heads).  Core c handles batch c//2 and head group
c%2: it computes Q/K/V with the 512-column slice of the projection
weights, runs attention for its 8 heads, and produces a partial output
projection through the matching 512-row slice of W_o.  The host sums the
two partials per batch and adds the constant bias term (bv @ Wo^T + bo).

Final kernel layout (669.7us baseline -> ~366us):
  - All matmul operands are fp16 (full-rate, FWL weight loads overlap the
    previous matmul) except the attn@V pass which runs fp8e4m3 DoubleRow
    (2 k-position chunks per instruction).  PSUM accumulation is fp32.
  - x^T and all weights are pre-laid-out on the host ([p, chunk, free])
    and DMA'd once into SBUF residents.
  - K bias is dropped entirely (softmax is invariant to per-query score
    shifts); Q bias is applied by the DVE during the PSUM->SBUF copy.
  - Scores for the two heads of an e-tile run as K=64 matmuls on PE row
    groups 0-63 / 64-127 concurrently (tile_position row tiling).
  - exp() on ACT writes fp8 eps directly; softmax denominator comes from
    a ones column appended to V; 1/denom via reciprocal_approx_fast
    (staged through SBUF - the custom op misreads PSUM).
  - The kernel is ACT(exp)-bound: the main emission loop carries only the
    score matmuls + exp.  Everything else (V projection, Q/K projections,
    attn@V, normalization, output projection) is drip-fed from a FIFO work
    queue a few instructions per kt step, so the exp stream starts ~7us in
    and never starves while the PE stays dense enough to keep the HAM
    clock-gate warm.
"""

import sys

for _p in ("/opt/trn_rl_repo",):
    if _p not in sys.path:
        sys.path.insert(0, _p)

import numpy as np
from collections import deque
from contextlib import ExitStack

import concourse.bass as bass
import concourse.mybir as mybir
import concourse.tile as tile
from concourse import bacc
from concourse.bass_utils import run_bass_kernel_spmd

F32 = mybir.dt.float32
F16 = mybir.dt.float16
F8 = mybir.dt.float8e4
AF = mybir.ActivationFunctionType
DR = mybir.MatmulPerfMode.DoubleRow

D, S = 1024, 2048   # d_model, seq len
E = 512             # per-core projection width (8 heads x 64)
H, DK = 8, 64       # heads per core, head dim
NB = D // 128       # contraction chunks (8)
SCALE = 0.125       # 1/sqrt(dk)


def build_bass(use_dr=True, pair_scores=True, fast_recip=True, eps_bufs=None,
               drain_et_boundary=True, ham_filler=0):
    if eps_bufs is None:
        # eps tiles must survive from their exp until the deferred attn@V
        # drains (up to ~3 qc later); fp8 tiles are 2KB/partition
        eps_bufs = 32 if use_dr else 16
    nc = bacc.Bacc(
        "TRN2", target_bir_lowering=False, debug=False, num_devices=8
    )
    xd = nc.dram_tensor("xd", [4, 128, NB, 512], F16, kind="ExternalInput").ap()
    wqd = nc.dram_tensor("wqd", [4, 128, NB, 128], F16, kind="ExternalInput").ap()
    wkd = nc.dram_tensor("wkd", [4, 128, NB, 128], F16, kind="ExternalInput").ap()
    wvd = nc.dram_tensor("wvd", [128, NB, E], F16, kind="ExternalInput").ap()
    wod = nc.dram_tensor("wod", [128, 4, D], F16, kind="ExternalInput").ap()
    bqd = nc.dram_tensor("bqd", [128, 4], F32, kind="ExternalInput").ap()
    y = nc.dram_tensor("y", [S, D], F32, kind="ExternalOutput").ap()

    with ExitStack() as ctx:
        tc = ctx.enter_context(tile.TileContext(nc))
        res = ctx.enter_context(tc.tile_pool(name="res", bufs=1))
        qkpool = ctx.enter_context(tc.tile_pool(name="qkpool", bufs=2))
        epool = ctx.enter_context(tc.tile_pool(name="epool", bufs=eps_bufs))
        drpool = ctx.enter_context(tc.tile_pool(name="drpool", bufs=2))
        bcpool = ctx.enter_context(tc.tile_pool(name="bcpool", bufs=2))
        ypool = ctx.enter_context(tc.tile_pool(name="ypool", bufs=2))
        ps_sp = ctx.enter_context(tc.tile_pool(name="ps_sp", bufs=2, space="PSUM"))
        ps_o = ctx.enter_context(tc.tile_pool(name="ps_o", bufs=2, space="PSUM"))
        ps_p = ctx.enter_context(tc.tile_pool(name="ps_p", bufs=2, space="PSUM"))

        # ---- residents ----
        xsb = res.tile([128, 4, NB, 512], F16, tag="x", name="xsb")
        wqt = res.tile([128, 4, NB, 128], F16, tag="wq", name="wqt")
        wkt = res.tile([128, 4, NB, 128], F16, tag="wk", name="wkt")
        wvt = res.tile([128, NB, E], F16, tag="wv", name="wvt")
        wot = res.tile([128, 4, D], F16, tag="wo", name="wot")
        bqt = res.tile([128, 4], F32, tag="bq", name="bqt")
        # V resident: per kt-pair, [p, pair, head, 80] fp8 (col 64 = ones,
        # cols 65-79 pad so the DoubleRow k-subtile step is 16B-aligned)
        vdt = F8 if use_dr else F16
        vt = [
            res.tile([128, 2, H, 80], vdt, tag="vt", bufs=8, name=f"vt{i}")
            for i in range(8)
        ]
        ao = [
            res.tile([128, S], F16, tag="ao", bufs=4, name=f"ao{i}")
            for i in range(4)
        ]
        # qT/kT double-buffered across e-tiles
        qT = [qkpool.tile([128, S], F16, tag="qT", name=f"qT{j}") for j in range(2)]
        kT = [qkpool.tile([128, S], F16, tag="kT", name=f"kT{j}") for j in range(2)]

        # DMA in consumption order: the prologue (Q/K e-tile 0, x columns
        # 0-511) only waits on ~1.5MB; V weights next so the V-projection
        # drip unblocks early; the rest streams in behind.
        nc.sync.dma_start(bqt[:, :], bqd[:, :])
        nc.sync.dma_start(wqt[:, 0, :, :], wqd[0])
        nc.sync.dma_start(wkt[:, 0, :, :], wkd[0])
        nc.sync.dma_start(xsb[:, 0, :, :], xd[0])
        nc.sync.dma_start(wvt[:, :, :], wvd[:, :, :])
        nc.sync.dma_start(xsb[:, 1, :, :], xd[1])
        for et_ in range(1, 4):
            nc.sync.dma_start(wqt[:, et_, :, :], wqd[et_])
            nc.sync.dma_start(wkt[:, et_, :, :], wkd[et_])
        nc.sync.dma_start(xsb[:, 2, :, :], xd[2])
        nc.sync.dma_start(xsb[:, 3, :, :], xd[3])
        nc.sync.dma_start(wot[:, :, :], wod[:, :, :])
        for i in range(8):
            nc.vector.memset(vt[i][:, :, :, 64:65], 1.0)

        # ---------- background work queue (drip-fed between kt steps) ----
        bg = deque()

        def push_q_chunk(et, sc):
            sl_ = slice(sc * 512, (sc + 1) * 512)
            st = {}

            def mm(dc):
                def f():
                    if "p" not in st:
                        st["p"] = ps_p.tile(
                            [128, 512], F32, tag="p", name=f"qp{et}_{sc}"
                        )
                    nc.tensor.matmul(
                        st["p"][:, :], wqt[:, et, dc, :], xsb[:, sc, dc, :],
                        start=(dc == 0), stop=(dc == NB - 1),
                    )
                return f

            for dc in range(NB):
                bgpush(mm(dc))
            bgpush(
                lambda: nc.vector.tensor_scalar_add(
                    qT[et % 2][:, sl_], st["p"][:, :], bqt[:, et : et + 1]
                )
            )

        def push_k_chunk(et, sc):
            sl_ = slice(sc * 512, (sc + 1) * 512)
            st = {}

            def mm(dc):
                def f():
                    if "p" not in st:
                        st["p"] = ps_p.tile(
                            [128, 512], F32, tag="p", name=f"kp{et}_{sc}"
                        )
                    nc.tensor.matmul(
                        st["p"][:, :], wkt[:, et, dc, :], xsb[:, sc, dc, :],
                        start=(dc == 0), stop=(dc == NB - 1),
                    )
                return f

            for dc in range(NB):
                bgpush(mm(dc))
            bgpush(
                lambda: nc.vector.tensor_copy(kT[et % 2][:, sl_], st["p"][:, :])
            )

        def push_v_chunk(tp, par):
            s_abs = 2 * tp + par
            st = {}

            def mm(dc):
                def f():
                    if "p" not in st:
                        st["p"] = ps_p.tile(
                            [128, 512], F32, tag="p", name=f"vp{s_abs}"
                        )
                    nc.tensor.matmul(
                        st["p"][:, :],
                        xsb[:, s_abs // 4, dc,
                            (s_abs % 4) * 128 : (s_abs % 4 + 1) * 128],
                        wvt[:, dc, :],
                        start=(dc == 0), stop=(dc == NB - 1),
                    )
                return f

            for dc in range(NB):
                bgpush(mm(dc))
            bgpush(
                lambda: nc.vector.tensor_copy(
                    vt[tp][:, par, :, 0:64],
                    st["p"].rearrange("p (h d) -> p h d", h=H),
                )
            )

        def push_attn_tail(et, qc, eps_list):
            """attn@V accumulation + softmax normalization for one qc."""
            qsl = slice(qc * 512, (qc + 1) * 512)
            st = {}

            def attnv(tp):
                def f():
                    if "o" not in st:
                        st["o"] = [
                            ps_o.tile(
                                [65, 512], F32, tag="o", name=f"o{et}_{qc}_{h}"
                            )
                            for h in range(2)
                        ]
                    ep = eps_list[tp]
                    for h in range(2):
                        hg = 2 * et + h
                        if use_dr:
                            nc.tensor.matmul(
                                st["o"][h][:, :],
                                vt[tp][:, :, hg, 0:65],
                                ep[:, h, :, :],
                                start=(tp == 0), stop=(tp == 7),
                                perf_mode=DR,
                            )
                        else:
                            for p2 in range(2):
                                nc.tensor.matmul(
                                    st["o"][h][:, :],
                                    vt[tp][:, p2, hg, 0:65],
                                    ep[:, h, p2, :],
                                    start=(tp == 0 and p2 == 0),
                                    stop=(tp == 7 and p2 == 1),
                                )
                return f

            for tp in range(8):
                bgpush(attnv(tp))

            def norm(h):
                def f():
                    o_h = st["o"][h]
                    dr_t = drpool.tile(
                        [1, 512], F32, tag="dr", name=f"dr{et}_{qc}_{h}"
                    )
                    if fast_recip:
                        dcp = drpool.tile(
                            [1, 512], F32, tag="dcp", name=f"dcp{et}_{qc}_{h}"
                        )
                        nc.vector.tensor_copy(dcp[:, :], o_h[64:65, :])
                        nc.vector.reciprocal_approx_fast(dr_t[:, :], dcp[:, :])
                    else:
                        nc.vector.reciprocal(dr_t[:, :], o_h[64:65, :])
                    bc_t = bcpool.tile(
                        [64, 512], F32, tag="bc", name=f"bc{et}_{qc}_{h}"
                    )
                    nc.gpsimd.partition_broadcast(bc_t[:, :], dr_t[:, :])
                    nc.vector.tensor_mul(
                        ao[et][h * 64 : (h + 1) * 64, qsl],
                        o_h[0:64, :],
                        bc_t[:, :],
                    )
                return f

            for h in range(2):
                bgpush(norm(h))

        def push_yproj(qt):
            qtsl = slice(qt * 128, (qt + 1) * 128)
            st = {}

            def mm(ec, oc):
                def f():
                    if oc not in st:
                        st[oc] = ps_p.tile(
                            [128, 512], F32, tag="p", name=f"yp{qt}_{oc}"
                        )
                    nc.tensor.matmul(
                        st[oc][:, :],
                        ao[ec][:, qtsl],
                        wot[:, ec, oc * 512 : (oc + 1) * 512],
                        start=(ec == 0), stop=(ec == 3),
                    )
                return f

            for ec in range(4):
                for oc in range(2):
                    bgpush(mm(ec, oc))

            def fin():
                ysb = ypool.tile([128, D], F32, tag="y", name=f"ysb{qt}")
                for oc in range(2):
                    nc.vector.tensor_copy(
                        ysb[:, oc * 512 : (oc + 1) * 512], st[oc][:, :]
                    )
                nc.sync.dma_start(y[qtsl, :], ysb[:, :])

            bgpush(fin)

        pushed = [0]
        drained = [0]

        def bgpush(item):
            bg.append(item)
            pushed[0] += 1

        def drain(n):
            for _ in range(min(n, len(bg))):
                bg.popleft()()
                drained[0] += 1

        def drain_until(mark):
            while drained[0] < mark and bg:
                bg.popleft()()
                drained[0] += 1

        def drain_all():
            while bg:
                bg.popleft()()
                drained[0] += 1

        # ---- prologue: just enough Q/K for (et0, qc0, kt0..3) ----
        k_marks, q_marks = {}, {}
        push_q_chunk(0, 0)
        q_marks[(0, 0)] = pushed[0]
        push_k_chunk(0, 0)
        k_marks[(0, 0)] = pushed[0]
        drain_all()
        # rest of K first (needed progressively by qc0's kt sweep), then V
        # (needed by the deferred attn@V), then the remaining Q chunks
        # (needed at qc1 start).
        for sc in range(1, 4):
            push_k_chunk(0, sc)
            k_marks[(0, sc)] = pushed[0]
        push_q_chunk(0, 1)
        q_marks[(0, 1)] = pushed[0]
        for tp in range(8):
            for par in range(2):
                push_v_chunk(tp, par)
        for sc in range(2, 4):
            push_q_chunk(0, sc)
            q_marks[(0, sc)] = pushed[0]

        # ---- main loop: scores + exp inline; all else drip-fed ----
        tail_marks = {}
        for et in range(4):
            qTe, kTe = qT[et % 2], kT[et % 2]
            for qc in range(4):
                # eps-pool safety: the attn@V consumers of the qc whose eps
                # buffers this qc's exps will overwrite must be emitted first
                gqc = 4 * et + qc
                if gqc - 3 in tail_marks:
                    drain_until(tail_marks[gqc - 3])
                # this qc's Q chunk must be emitted before its scores
                drain_until(q_marks.get((et, qc), 0))
                if et < 3:
                    push_k_chunk(et + 1, qc)
                    k_marks[(et + 1, qc)] = pushed[0]
                    push_q_chunk(et + 1, qc)
                    q_marks[(et + 1, qc)] = pushed[0]
                else:
                    if qc > 0:
                        for qt in range(4 * (qc - 1), 4 * qc):
                            push_yproj(qt)
                qsl = slice(qc * 512, (qc + 1) * 512)
                eps_list = []
                for kt in range(16):
                    if kt % 4 == 0:
                        # the K chunk covering this kt range must be emitted
                        drain_until(k_marks.get((et, kt // 4), 0))
                    # adaptive drip rate: finish this e-tile's backlog a
                    # couple of kt steps before its boundary so the forced
                    # drain never bursts; skip the drip entirely in the DMA
                    # shadow at the very start and on each qc's last kt
                    kts_left = (4 - qc) * 16 - kt - 2
                    if et == 0 and qc == 0 and kt < 5:
                        rate = 0
                    elif kt >= 15:
                        rate = 0
                    else:
                        rate = max(3, min(8, -(-len(bg) // max(kts_left, 1))))
                        # the eps barrier that fires at the NEXT qc must not
                        # burst: pace toward its mark within this qc
                        dl = tail_marks.get(gqc - (eps_bufs // 8 - 2), 0) - drained[0]
                        if dl > 0:
                            rate = max(rate, min(10, -(-dl // max(14 - kt, 1))))
                    ksl = slice(kt * 128, (kt + 1) * 128)
                    sp = ps_sp.tile(
                        [128, 1024], F32, tag="sp", name=f"sp{et}_{qc}_{kt}"
                    )
                    if pair_scores:
                        nc.tensor.matmul(
                            sp[:, 0:512], kTe[0:64, ksl], qTe[0:64, qsl],
                            start=True, stop=True, tile_position=(0, 0),
                        )
                        nc.tensor.matmul(
                            sp[:, 512:1024], kTe[64:128, ksl], qTe[64:128, qsl],
                            start=True, stop=True, tile_position=(64, 0),
                        )
                    else:
                        for h in range(2):
                            nc.tensor.matmul(
                                sp[:, h * 512 : (h + 1) * 512],
                                kTe[h * 64 : h * 64 + 64, ksl],
                                qTe[h * 64 : h * 64 + 64, qsl],
                                start=True, stop=True,
                                tile_position=(64 * h, 0),
                            )
                    par = kt % 2
                    if par == 0:
                        ep = epool.tile(
                            [128, 2, 2, 512], F8 if use_dr else F16,
                            tag="eps", name=f"ep{et}_{qc}_{kt // 2}",
                        )
                        eps_list.append(ep)
                    # one exp covers both heads; out strided [h, par, q]
                    nc.scalar.activation(
                        ep[:, :, par, :],
                        sp.rearrange("p (h q) -> p h q", h=2),
                        AF.Exp,
                        scale=SCALE,
                    )
                    if par == 1 and et == 3 and qc == 3:
                        # very last qc: attn@V inline so only the output
                        # projection trails the final exp
                        tp = kt // 2
                        if tp == 0:
                            drain_until(tail_marks[14])
                            st_last = [
                                ps_o.tile(
                                    [65, 512], F32, tag="o", name=f"o3_3_{h}"
                                )
                                for h in range(2)
                            ]
                        for h in range(2):
                            hg = 6 + h
                            if use_dr:
                                nc.tensor.matmul(
                                    st_last[h][:, :],
                                    vt[tp][:, :, hg, 0:65],
                                    ep[:, h, :, :],
                                    start=(tp == 0), stop=(tp == 7),
                                    perf_mode=DR,
                                )
                            else:
                                for p2 in range(2):
                                    nc.tensor.matmul(
                                        st_last[h][:, :],
                                        vt[tp][:, p2, hg, 0:65],
                                        ep[:, h, p2, :],
                                        start=(tp == 0 and p2 == 0),
                                        stop=(tp == 7 and p2 == 1),
                                    )
                    drain(rate)
                    # optional: keep the PE activity monitor fed so the
                    # HAM clock gate stays at 8/8 during ACT-bound stretches
                    for _ in range(ham_filler if et >= 1 else 0):
                        nc.tensor.ldweights(wvt[:, 0, 0:128])
                if et == 3 and qc == 3:
                    for h in range(2):
                        dr_t = drpool.tile([1, 512], F32, tag="dr", name=f"dr33_{h}")
                        if fast_recip:
                            dcp = drpool.tile(
                                [1, 512], F32, tag="dcp", name=f"dcp33_{h}"
                            )
                            nc.vector.tensor_copy(dcp[:, :], st_last[h][64:65, :])
                            nc.vector.reciprocal_approx_fast(dr_t[:, :], dcp[:, :])
                        else:
                            nc.vector.reciprocal(dr_t[:, :], st_last[h][64:65, :])
                        bc_t = bcpool.tile([64, 512], F32, tag="bc", name=f"bc33_{h}")
                        nc.gpsimd.partition_broadcast(bc_t[:, :], dr_t[:, :])
                        nc.vector.tensor_mul(
                            ao[3][h * 64 : (h + 1) * 64, qsl],
                            st_last[h][0:64, :],
                            bc_t[:, :],
                        )
                else:
                    push_attn_tail(et, qc, eps_list)
                    tail_marks[gqc] = pushed[0]
            if drain_et_boundary and et < 3:
                drain_all()
        for qt in range(12, 16):
            push_yproj(qt)
        drain_all()

    nc.finalize()
    return nc


def make_in_maps(x, Wq, Wk, Wv, Wo, bq):
    def chunked(w):  # [D, n] -> [128, D//128, n]
        n = w.shape[1]
        return np.ascontiguousarray(
            w.reshape(-1, 128, n).transpose(1, 0, 2), dtype=np.float16
        )

    def blocked(w, nblk):  # [128, NB, n] -> [nblk, 128, NB, n//nblk]
        n = w.shape[2]
        return np.ascontiguousarray(
            w.reshape(128, NB, nblk, n // nblk).transpose(2, 0, 1, 3)
        )

    in_maps = []
    for c in range(8):
        b, g = divmod(c, 2)
        sl = slice(g * E, (g + 1) * E)
        in_maps.append(
            {
                "xd": blocked(chunked(x[b].T), 4),      # [4, 128, 8, 512]
                "wqd": blocked(chunked(Wq[sl, :].T), 4),  # [4, 128, 8, 128]
                "wkd": blocked(chunked(Wk[sl, :].T), 4),
                "wvd": chunked(Wv[sl, :].T),
                "wod": chunked(Wo[:, sl].T),           # [128, 4, D]
                "bqd": np.ascontiguousarray(
                    bq[sl].reshape(4, 128).T, dtype=np.float32
                ),
            }
        )
    return in_maps


_NC = None


def run(x, Wq, bq, Wk, bk, Wv, bv, Wo, bo, build_kwargs=None, **run_kwargs):
    global _NC
    x = np.asarray(x, dtype=np.float32)
    Wq, Wk, Wv, Wo = (np.asarray(a, dtype=np.float32) for a in (Wq, Wk, Wv, Wo))
    bq, bk, bv, bo = (np.asarray(a, dtype=np.float32) for a in (bq, bk, bv, bo))
    if _NC is None:
        _NC = build_bass(**(build_kwargs or {}))
    in_maps = make_in_maps(x, Wq, Wk, Wv, Wo, bq)
    try:
        res = run_bass_kernel_spmd(
            _NC, in_maps, core_ids=list(range(8)), **run_kwargs
        )
    except Exception:
        # One retry: a previously wedged device can fail the first attempt.
        res = run_bass_kernel_spmd(
            _NC, in_maps, core_ids=list(range(8)), **run_kwargs
        )
    ys = [r["y"] for r in res.results]
    c_vec = (bv @ Wo.T + bo).astype(np.float32)  # constant bias fold
    out = np.stack([ys[2 * b] + ys[2 * b + 1] + c_vec for b in range(4)])
    return out.astype(np.float32), res


def kernel(x, Wq, bq, Wk, bk, Wv, bv, Wo, bo):
    out, _ = run(x, Wq, bq, Wk, bk, Wv, bv, Wo, bo)
    return out


# revision 30
# speedup vs baseline: 1.0092x; 1.0092x over previous
"""Multi-head attention (B=4, S=2048, D=1024, H=16, dk=64) on 8 TRN2 cores.

Sharding: data-parallel over B (4 batches) x tensor-parallel over head
groups (2 groups of 8 heads).  Core c handles batch c//2 and head group
c%2: it computes Q/K/V with the 512-column slice of the projection
weights, runs attention for its 8 heads, and produces a partial output
projection through the matching 512-row slice of W_o.  The host sums the
two partials per batch and adds the constant bias term (bv @ Wo^T + bo).

Final kernel layout (669.7us baseline -> ~366us):
  - All matmul operands are fp16 (full-rate, FWL weight loads overlap the
    previous matmul) except the attn@V pass which runs fp8e4m3 DoubleRow
    (2 k-position chunks per instruction).  PSUM accumulation is fp32.
  - x^T and all weights are pre-laid-out on the host ([p, chunk, free])
    and DMA'd once into SBUF residents.
  - K bias is dropped entirely (softmax is invariant to per-query score
    shifts); Q bias is applied by the DVE during the PSUM->SBUF copy.
  - Scores for the two heads of an e-tile run as K=64 matmuls on PE row
    groups 0-63 / 64-127 concurrently (tile_position row tiling).
  - exp() on ACT writes fp8 eps directly; softmax denominator comes from
    a ones column appended to V; 1/denom via reciprocal_approx_fast
    (staged through SBUF - the custom op misreads PSUM).
  - The kernel is ACT(exp)-bound: the main emission loop carries only the
    score matmuls + exp.  Everything else (V projection, Q/K projections,
    attn@V, normalization, output projection) is drip-fed from a FIFO work
    queue a few instructions per kt step, so the exp stream starts ~7us in
    and never starves while the PE stays dense enough to keep the HAM
    clock-gate warm.
"""

import sys

for _p in ("/opt/trn_rl_repo",):
    if _p not in sys.path:
        sys.path.insert(0, _p)

import numpy as np
from collections import deque
from contextlib import ExitStack

import concourse.bass as bass
import concourse.mybir as mybir
import concourse.tile as tile
from concourse import bacc
from concourse.bass_utils import run_bass_kernel_spmd

F32 = mybir.dt.float32
F16 = mybir.dt.float16
F8 = mybir.dt.float8e4
AF = mybir.ActivationFunctionType
DR = mybir.MatmulPerfMode.DoubleRow

D, S = 1024, 2048   # d_model, seq len
E = 512             # per-core projection width (8 heads x 64)
H, DK = 8, 64       # heads per core, head dim
NB = D // 128       # contraction chunks (8)
SCALE = 0.125       # 1/sqrt(dk)


def build_bass(use_dr=True, pair_scores=True, fast_recip=True, eps_bufs=None,
               drain_et_boundary=True, ham_filler=0):
    if eps_bufs is None:
        # eps tiles must survive from their exp until the deferred attn@V
        # drains (up to ~3 qc later); fp8 tiles are 2KB/partition
        eps_bufs = 32 if use_dr else 16
    nc = bacc.Bacc(
        "TRN2", target_bir_lowering=False, debug=False, num_devices=8
    )
    xd = nc.dram_tensor("xd", [4, 128, NB, 512], F16, kind="ExternalInput").ap()
    wqd = nc.dram_tensor("wqd", [4, 128, NB, 128], F16, kind="ExternalInput").ap()
    wkd = nc.dram_tensor("wkd", [4, 128, NB, 128], F16, kind="ExternalInput").ap()
    wvd = nc.dram_tensor("wvd", [128, NB, E], F16, kind="ExternalInput").ap()
    wod = nc.dram_tensor("wod", [128, 4, D], F16, kind="ExternalInput").ap()
    bqd = nc.dram_tensor("bqd", [128, 4], F32, kind="ExternalInput").ap()
    y = nc.dram_tensor("y", [S, D], F32, kind="ExternalOutput").ap()

    with ExitStack() as ctx:
        tc = ctx.enter_context(tile.TileContext(nc))
        res = ctx.enter_context(tc.tile_pool(name="res", bufs=1))
        qkpool = ctx.enter_context(tc.tile_pool(name="qkpool", bufs=2))
        epool = ctx.enter_context(tc.tile_pool(name="epool", bufs=eps_bufs))
        drpool = ctx.enter_context(tc.tile_pool(name="drpool", bufs=2))
        bcpool = ctx.enter_context(tc.tile_pool(name="bcpool", bufs=2))
        ypool = ctx.enter_context(tc.tile_pool(name="ypool", bufs=2))
        ps_sp = ctx.enter_context(tc.tile_pool(name="ps_sp", bufs=2, space="PSUM"))
        ps_o = ctx.enter_context(tc.tile_pool(name="ps_o", bufs=2, space="PSUM"))
        ps_p = ctx.enter_context(tc.tile_pool(name="ps_p", bufs=2, space="PSUM"))

        # ---- residents ----
        xsb = res.tile([128, 4, NB, 512], F16, tag="x", name="xsb")
        wqt = res.tile([128, 4, NB, 128], F16, tag="wq", name="wqt")
        wkt = res.tile([128, 4, NB, 128], F16, tag="wk", name="wkt")
        wvt = res.tile([128, NB, E], F16, tag="wv", name="wvt")
        wot = res.tile([128, 4, D], F16, tag="wo", name="wot")
        bqt = res.tile([128, 4], F32, tag="bq", name="bqt")
        # V resident: per kt-pair, [p, pair, head, 80] fp8 (col 64 = ones,
        # cols 65-79 pad so the DoubleRow k-subtile step is 16B-aligned)
        vdt = F8 if use_dr else F16
        vt = [
            res.tile([128, 2, H, 80], vdt, tag="vt", bufs=8, name=f"vt{i}")
            for i in range(8)
        ]
        ao = [
            res.tile([128, S], F16, tag="ao", bufs=4, name=f"ao{i}")
            for i in range(4)
        ]
        # qT/kT double-buffered across e-tiles
        qT = [qkpool.tile([128, S], F16, tag="qT", name=f"qT{j}") for j in range(2)]
        kT = [qkpool.tile([128, S], F16, tag="kT", name=f"kT{j}") for j in range(2)]

        # DMA in consumption order: the prologue (Q/K e-tile 0, x columns
        # 0-511) only waits on ~1.5MB; V weights next so the V-projection
        # drip unblocks early; the rest streams in behind.
        nc.sync.dma_start(bqt[:, :], bqd[:, :])
        nc.sync.dma_start(wqt[:, 0, :, :], wqd[0])
        nc.sync.dma_start(wkt[:, 0, :, :], wkd[0])
        nc.sync.dma_start(xsb[:, 0, :, :], xd[0])
        nc.sync.dma_start(wvt[:, :, :], wvd[:, :, :])
        nc.sync.dma_start(xsb[:, 1, :, :], xd[1])
        for et_ in range(1, 4):
            nc.sync.dma_start(wqt[:, et_, :, :], wqd[et_])
            nc.sync.dma_start(wkt[:, et_, :, :], wkd[et_])
        nc.sync.dma_start(xsb[:, 2, :, :], xd[2])
        nc.sync.dma_start(xsb[:, 3, :, :], xd[3])
        nc.sync.dma_start(wot[:, :, :], wod[:, :, :])
        for i in range(8):
            nc.vector.memset(vt[i][:, :, :, 64:65], 1.0)

        # ---------- background work queue (drip-fed between kt steps) ----
        bg = deque()

        def push_q_chunk(et, sc):
            sl_ = slice(sc * 512, (sc + 1) * 512)
            st = {}

            def mm(dc):
                def f():
                    if "p" not in st:
                        st["p"] = ps_p.tile(
                            [128, 512], F32, tag="p", name=f"qp{et}_{sc}"
                        )
                    nc.tensor.matmul(
                        st["p"][:, :], wqt[:, et, dc, :], xsb[:, sc, dc, :],
                        start=(dc == 0), stop=(dc == NB - 1),
                    )
                return f

            for dc in range(NB):
                bgpush(mm(dc))
            bgpush(
                lambda: nc.vector.tensor_scalar_add(
                    qT[et % 2][:, sl_], st["p"][:, :], bqt[:, et : et + 1]
                )
            )

        def push_k_chunk(et, sc):
            sl_ = slice(sc * 512, (sc + 1) * 512)
            st = {}

            def mm(dc):
                def f():
                    if "p" not in st:
                        st["p"] = ps_p.tile(
                            [128, 512], F32, tag="p", name=f"kp{et}_{sc}"
                        )
                    nc.tensor.matmul(
                        st["p"][:, :], wkt[:, et, dc, :], xsb[:, sc, dc, :],
                        start=(dc == 0), stop=(dc == NB - 1),
                    )
                return f

            for dc in range(NB):
                bgpush(mm(dc))
            bgpush(
                lambda: nc.vector.tensor_copy(kT[et % 2][:, sl_], st["p"][:, :])
            )

        def push_v_chunk(tp, par):
            s_abs = 2 * tp + par
            st = {}

            def mm(dc):
                def f():
                    if "p" not in st:
                        st["p"] = ps_p.tile(
                            [128, 512], F32, tag="p", name=f"vp{s_abs}"
                        )
                    nc.tensor.matmul(
                        st["p"][:, :],
                        xsb[:, s_abs // 4, dc,
                            (s_abs % 4) * 128 : (s_abs % 4 + 1) * 128],
                        wvt[:, dc, :],
                        start=(dc == 0), stop=(dc == NB - 1),
                    )
                return f

            for dc in range(NB):
                bgpush(mm(dc))
            bgpush(
                lambda: nc.vector.tensor_copy(
                    vt[tp][:, par, :, 0:64],
                    st["p"].rearrange("p (h d) -> p h d", h=H),
                )
            )

        def push_attn_tail(et, qc, eps_list):
            """attn@V accumulation + softmax normalization for one qc."""
            qsl = slice(qc * 512, (qc + 1) * 512)
            st = {}

            def attnv(tp):
                def f():
                    if "o" not in st:
                        st["o"] = [
                            ps_o.tile(
                                [65, 512], F32, tag="o", name=f"o{et}_{qc}_{h}"
                            )
                            for h in range(2)
                        ]
                    ep = eps_list[tp]
                    for h in range(2):
                        hg = 2 * et + h
                        if use_dr:
                            nc.tensor.matmul(
                                st["o"][h][:, :],
                                vt[tp][:, :, hg, 0:65],
                                ep[:, h, :, :],
                                start=(tp == 0), stop=(tp == 7),
                                perf_mode=DR,
                            )
                        else:
                            for p2 in range(2):
                                nc.tensor.matmul(
                                    st["o"][h][:, :],
                                    vt[tp][:, p2, hg, 0:65],
                                    ep[:, h, p2, :],
                                    start=(tp == 0 and p2 == 0),
                                    stop=(tp == 7 and p2 == 1),
                                )
                return f

            for tp in range(8):
                bgpush(attnv(tp))

            def norm(h):
                def f():
                    o_h = st["o"][h]
                    dr_t = drpool.tile(
                        [1, 512], F32, tag="dr", name=f"dr{et}_{qc}_{h}"
                    )
                    if fast_recip:
                        dcp = drpool.tile(
                            [1, 512], F32, tag="dcp", name=f"dcp{et}_{qc}_{h}"
                        )
                        nc.vector.tensor_copy(dcp[:, :], o_h[64:65, :])
                        nc.vector.reciprocal_approx_fast(dr_t[:, :], dcp[:, :])
                    else:
                        nc.vector.reciprocal(dr_t[:, :], o_h[64:65, :])
                    bc_t = bcpool.tile(
                        [64, 512], F32, tag="bc", name=f"bc{et}_{qc}_{h}"
                    )
                    nc.gpsimd.partition_broadcast(bc_t[:, :], dr_t[:, :])
                    nc.vector.tensor_mul(
                        ao[et][h * 64 : (h + 1) * 64, qsl],
                        o_h[0:64, :],
                        bc_t[:, :],
                    )
                return f

            for h in range(2):
                bgpush(norm(h))

        def push_yproj(qt):
            qtsl = slice(qt * 128, (qt + 1) * 128)
            st = {}

            def mm(ec, oc):
                def f():
                    if oc not in st:
                        st[oc] = ps_p.tile(
                            [128, 512], F32, tag="p", name=f"yp{qt}_{oc}"
                        )
                    nc.tensor.matmul(
                        st[oc][:, :],
                        ao[ec][:, qtsl],
                        wot[:, ec, oc * 512 : (oc + 1) * 512],
                        start=(ec == 0), stop=(ec == 3),
                    )
                return f

            for ec in range(4):
                for oc in range(2):
                    bgpush(mm(ec, oc))

            def fin():
                ysb = ypool.tile([128, D], F32, tag="y", name=f"ysb{qt}")
                for oc in range(2):
                    nc.vector.tensor_copy(
                        ysb[:, oc * 512 : (oc + 1) * 512], st[oc][:, :]
                    )
                nc.sync.dma_start(y[qtsl, :], ysb[:, :])

            bgpush(fin)

        pushed = [0]
        drained = [0]

        def bgpush(item):
            bg.append(item)
            pushed[0] += 1

        def drain(n):
            for _ in range(min(n, len(bg))):
                bg.popleft()()
                drained[0] += 1

        def drain_until(mark):
            while drained[0] < mark and bg:
                bg.popleft()()
                drained[0] += 1

        def drain_all():
            while bg:
                bg.popleft()()
                drained[0] += 1

        # ---- prologue: just enough Q/K for (et0, qc0, kt0..3) ----
        k_marks, q_marks = {}, {}
        push_q_chunk(0, 0)
        q_marks[(0, 0)] = pushed[0]
        push_k_chunk(0, 0)
        k_marks[(0, 0)] = pushed[0]
        drain_all()
        # rest of K first (needed progressively by qc0's kt sweep), then V
        # (needed by the deferred attn@V), then the remaining Q chunks
        # (needed at qc1 start).
        for sc in range(1, 4):
            push_k_chunk(0, sc)
            k_marks[(0, sc)] = pushed[0]
        push_q_chunk(0, 1)
        q_marks[(0, 1)] = pushed[0]
        for tp in range(8):
            for par in range(2):
                push_v_chunk(tp, par)
        for sc in range(2, 4):
            push_q_chunk(0, sc)
            q_marks[(0, sc)] = pushed[0]

        # ---- main loop: scores + exp inline; all else drip-fed ----
        tail_marks = {}
        for et in range(4):
            qTe, kTe = qT[et % 2], kT[et % 2]
            for qc in range(4):
                # eps-pool safety: the attn@V consumers of the qc whose eps
                # buffers this qc's exps will overwrite must be emitted first
                gqc = 4 * et + qc
                if gqc - 3 in tail_marks:
                    drain_until(tail_marks[gqc - 3])
                # this qc's Q chunk must be emitted before its scores
                drain_until(q_marks.get((et, qc), 0))
                if et < 3:
                    push_k_chunk(et + 1, qc)
                    k_marks[(et + 1, qc)] = pushed[0]
                    push_q_chunk(et + 1, qc)
                    q_marks[(et + 1, qc)] = pushed[0]
                else:
                    if qc > 0:
                        for qt in range(4 * (qc - 1), 4 * qc):
                            push_yproj(qt)
                qsl = slice(qc * 512, (qc + 1) * 512)
                eps_list = []
                for kt in range(16):
                    if kt % 4 == 0:
                        # the K chunk covering this kt range must be emitted
                        drain_until(k_marks.get((et, kt // 4), 0))
                    # adaptive drip rate: finish this e-tile's backlog a
                    # couple of kt steps before its boundary so the forced
                    # drain never bursts; skip the drip entirely in the DMA
                    # shadow at the very start and on each qc's last kt
                    kts_left = (4 - qc) * 16 - kt - 2
                    if et == 0 and qc == 0 and kt < 5:
                        rate = 0
                    elif kt >= 15:
                        rate = 0
                    else:
                        rate = max(3, min(8, -(-len(bg) // max(kts_left, 1))))
                        # the eps barrier that fires at the NEXT qc must not
                        # burst: pace toward its mark within this qc
                        dl = tail_marks.get(gqc - (eps_bufs // 8 - 2), 0) - drained[0]
                        if dl > 0:
                            rate = max(rate, min(10, -(-dl // max(14 - kt, 1))))
                    ksl = slice(kt * 128, (kt + 1) * 128)
                    sp = ps_sp.tile(
                        [128, 1024], F32, tag="sp", name=f"sp{et}_{qc}_{kt}"
                    )
                    if pair_scores:
                        nc.tensor.matmul(
                            sp[:, 0:512], kTe[0:64, ksl], qTe[0:64, qsl],
                            start=True, stop=True, tile_position=(0, 0),
                        )
                        nc.tensor.matmul(
                            sp[:, 512:1024], kTe[64:128, ksl], qTe[64:128, qsl],
                            start=True, stop=True, tile_position=(64, 0),
                        )
                    else:
                        for h in range(2):
                            nc.tensor.matmul(
                                sp[:, h * 512 : (h + 1) * 512],
                                kTe[h * 64 : h * 64 + 64, ksl],
                                qTe[h * 64 : h * 64 + 64, qsl],
                                start=True, stop=True,
                                tile_position=(64 * h, 0),
                            )
                    par = kt % 2
                    if par == 0:
                        ep = epool.tile(
                            [128, 2, 2, 512], F8 if use_dr else F16,
                            tag="eps", name=f"ep{et}_{qc}_{kt // 2}",
                        )
                        eps_list.append(ep)
                    # one exp covers both heads; out strided [h, par, q]
                    nc.scalar.activation(
                        ep[:, :, par, :],
                        sp.rearrange("p (h q) -> p h q", h=2),
                        AF.Exp,
                        scale=SCALE,
                    )
                    if par == 1 and et == 3 and qc == 3:
                        # very last qc: attn@V inline so only the output
                        # projection trails the final exp
                        tp = kt // 2
                        if tp == 0:
                            drain_until(tail_marks[14])
                            st_last = [
                                ps_o.tile(
                                    [65, 512], F32, tag="o", name=f"o3_3_{h}"
                                )
                                for h in range(2)
                            ]
                        for h in range(2):
                            hg = 6 + h
                            if use_dr:
                                nc.tensor.matmul(
                                    st_last[h][:, :],
                                    vt[tp][:, :, hg, 0:65],
                                    ep[:, h, :, :],
                                    start=(tp == 0), stop=(tp == 7),
                                    perf_mode=DR,
                                )
                            else:
                                for p2 in range(2):
                                    nc.tensor.matmul(
                                        st_last[h][:, :],
                                        vt[tp][:, p2, hg, 0:65],
                                        ep[:, h, p2, :],
                                        start=(tp == 0 and p2 == 0),
                                        stop=(tp == 7 and p2 == 1),
                                    )
                    drain(rate)
                    # optional: keep the PE activity monitor fed so the
                    # HAM clock gate stays at 8/8 during ACT-bound stretches
                    for _ in range(ham_filler if et >= 1 else 0):
                        nc.tensor.ldweights(wvt[:, 0, 0:128])
                if et == 3 and qc == 3:
                    for h in range(2):
                        dr_t = drpool.tile([1, 512], F32, tag="dr", name=f"dr33_{h}")
                        if fast_recip:
                            dcp = drpool.tile(
                                [1, 512], F32, tag="dcp", name=f"dcp33_{h}"
                            )
                            nc.vector.tensor_copy(dcp[:, :], st_last[h][64:65, :])
                            nc.vector.reciprocal_approx_fast(dr_t[:, :], dcp[:, :])
                        else:
                            nc.vector.reciprocal(dr_t[:, :], st_last[h][64:65, :])
                        bc_t = bcpool.tile([64, 512], F32, tag="bc", name=f"bc33_{h}")
                        nc.gpsimd.partition_broadcast(bc_t[:, :], dr_t[:, :])
                        nc.vector.tensor_mul(
                            ao[3][h * 64 : (h + 1) * 64, qsl],
                            st_last[h][0:64, :],
                            bc_t[:, :],
                        )
                else:
                    push_attn_tail(et, qc, eps_list)
                    tail_marks[gqc] = pushed[0]
            if drain_et_boundary and et < 3:
                drain_all()
        for qt in range(12, 16):
            push_yproj(qt)
        drain_all()

    nc.finalize()
    return nc


def make_in_maps(x, Wq, Wk, Wv, Wo, bq):
    def chunked(w):  # [D, n] -> [128, D//128, n]
        n = w.shape[1]
        return np.ascontiguousarray(
            w.reshape(-1, 128, n).transpose(1, 0, 2), dtype=np.float16
        )

    def blocked(w, nblk):  # [128, NB, n] -> [nblk, 128, NB, n//nblk]
        n = w.shape[2]
        return np.ascontiguousarray(
            w.reshape(128, NB, nblk, n // nblk).transpose(2, 0, 1, 3)
        )

    in_maps = []
    for c in range(8):
        b, g = divmod(c, 2)
        sl = slice(g * E, (g + 1) * E)
        in_maps.append(
            {
                "xd": blocked(chunked(x[b].T), 4),      # [4, 128, 8, 512]
                "wqd": blocked(chunked(Wq[sl, :].T), 4),  # [4, 128, 8, 128]
                "wkd": blocked(chunked(Wk[sl, :].T), 4),
                "wvd": chunked(Wv[sl, :].T),
                "wod": chunked(Wo[:, sl].T),           # [128, 4, D]
                "bqd": np.ascontiguousarray(
                    bq[sl].reshape(4, 128).T, dtype=np.float32
                ),
            }
        )
    return in_maps


_NC = None


def run(x, Wq, bq, Wk, bk, Wv, bv, Wo, bo, build_kwargs=None, **run_kwargs):
    global _NC
    x = np.asarray(x, dtype=np.float32)
    Wq, Wk, Wv, Wo = (np.asarray(a, dtype=np.float32) for a in (Wq, Wk, Wv, Wo))
    bq, bk, bv, bo = (np.asarray(a, dtype=np.float32) for a in (bq, bk, bv, bo))
    if _NC is None:
        _NC = build_bass(**(build_kwargs or {}))
    in_maps = make_in_maps(x, Wq, Wk, Wv, Wo, bq)
    try:
        res = run_bass_kernel_spmd(
            _NC, in_maps, core_ids=list(range(8)), **run_kwargs
        )
    except Exception:
        # One retry: a previously wedged device can fail the first attempt.
        res = run_bass_kernel_spmd(
            _NC, in_maps, core_ids=list(range(8)), **run_kwargs
        )
    ys = [r["y"] for r in res.results]
    c_vec = (bv @ Wo.T + bo).astype(np.float32)  # constant bias fold
    out = np.stack([ys[2 * b] + ys[2 * b + 1] + c_vec for b in range(4)])
    return out.astype(np.float32), res


def kernel(x, Wq, bq, Wk, bk, Wv, bv, Wo, bo):
    out, _ = run(x, Wq, bq, Wk, bk, Wv, bv, Wo, bo)
    return out
